# revision 1
# baseline (speedup 1.0000x reference)
"""GraphSAGE 2-layer forward on 8 Trainium2 NeuronCores (Bass/Tile).

Strategy (graph/data parallel, per sharding hint):
  - Destination nodes partitioned across 8 cores (12500 each); edges bucketed
    to the core owning their dst, grouped into 128-dst-node blocks.
  - Layer-0 messages are gathered in y0-space (y0 = x @ Wl0.T, 64-wide) so the
    per-edge payload is halved; y0 shards are exchanged with an AllGather
    ("halo exchange" degenerates to full exchange for random graphs).
  - Per-edge gather uses dma_gather with int16 indices over a 4-row-packed
    table (512B tokens, full DMA descriptor rate); a per-token mask*1/deg
    vector zeroes the 3 unwanted rows of each 4-pack and folds in the mean
    normalization.
  - Segment-sum over sorted-by-dst edges is computed on the TensorEngine:
    S[k, j] = (dst_rel[k] == j) one-hot built with iota + is_equal, then
    psum[feat, node] += msg.T @ S accumulated over 128-edge chunks.
  - Layer-1 repeats the pattern with a 4-packed fp32 y1 = h @ Wl1.T table
    (16-wide) after a second AllGather.
Weights are replicated; all dense matmuls keep features on partitions.
"""
import numpy as np
from contextlib import ExitStack

from concourse import bass, bacc, mybir, tile
from concourse.bass_utils import run_bass_kernel_spmd

dt = mybir.dt
NO_COLLECTIVE = False   # hang-bisect: replace AllGathers with local copies
PHASES = 6              # hang-bisect: build only phases <= this
P4LEVEL = 9             # hang-bisect: per-block work level in phase 4
SKIP12 = False          # hang-bisect: skip phases 1-2

# problem constants (hardcoded per harness contract)
N = 100000
FEAT = 128
EMB = 64
HID = 64
OUT = 16
NC_N = 8
NODES_PER = N // NC_N          # 12500
P = 128


def default_cfg(slots):
    blocks = (NODES_PER + P - 1) // P          # 98
    return dict(
        n=N, nodes_per=NODES_PER, blocks=blocks, slots=slots,
        chunks=slots // P, gb=1,               # blocks per gather group
        feat=FEAT, emb=EMB, hid=HID, out=OUT, nc_n=NC_N,
    )


def build_nc(cfg):
    n, nodes_per = cfg["n"], cfg["nodes_per"]
    blocks, slots, chunks, gb = cfg["blocks"], cfg["slots"], cfg["chunks"], cfg["gb"]
    feat, emb, hid, dout, nc_n = cfg["feat"], cfg["emb"], cfg["hid"], cfg["out"], cfg["nc_n"]
    ngroups = (blocks + gb - 1) // gb
    tok = slots * blocks                        # gather tokens per core
    gtok = slots * gb                           # tokens per gather call
    tail = nodes_per - (blocks - 1) * P         # nodes in last block

    nc = bacc.Bacc("TRN2", target_bir_lowering=False, debug=False, num_devices=nc_n)

    def din(name, shape, d):
        return nc.dram_tensor(name, shape, d, kind="ExternalInput").ap()

    # --- inputs (per core) ---
    xT = din("xT", [feat, nodes_per], dt.float32)          # own x, transposed
    xembT = din("xembT", [2 * emb, nodes_per], dt.float32)  # own emb, transposed
    srcp = din("srcp", [P, tok // 16], dt.int16)           # src//4, dma_gather layout
    dstrel = din("dstrel", [P, slots // P * blocks], dt.int16)
    maskrd = din("maskrd", [P, slots // P * blocks * 4], dt.float16)
    Wl0T = din("Wl0T", [feat, hid], dt.float32)
    Wr0T = din("Wr0T", [feat, hid], dt.float32)
    We0Th = din("We0Th", [emb, hid], dt.float32)           # We0.T * 0.5
    Wr1T = din("Wr1T", [hid, dout], dt.float16)
    We1T = din("We1T", [hid, dout], dt.float16)
    Wl1T = din("Wl1T", [hid, dout], dt.float16)
    b0 = din("b0", [hid, 1], dt.float32)                   # bl0
    be0 = din("be0", [hid, 1], dt.float32)
    b1 = din("b1", [dout, 1], dt.float32)                  # bl1 + be1
    i642 = din("i642", [P, hid], dt.float16)               # vstack(I64, I64)
    out_own = nc.dram_tensor("out_own", [nodes_per, dout], dt.float32,
                             kind="ExternalOutput").ap()

    cg = chunks * gb                                       # chunks per gather group

    with tile.TileContext(nc) as tc, ExitStack() as ctx:
        const = ctx.enter_context(tc.tile_pool(name="const", bufs=1))
        sb = ctx.enter_context(tc.tile_pool(name="sb", bufs=2))
        sb3 = ctx.enter_context(tc.tile_pool(name="sb3", bufs=3))
        pp = ctx.enter_context(tc.tile_pool(name="pp", bufs=2, space="PSUM"))
        pp1 = ctx.enter_context(tc.tile_pool(name="pp1", bufs=4, space="PSUM"))
        dram = ctx.enter_context(tc.tile_pool(name="dram", bufs=1, space="DRAM"))

        # --- resident constants ---
        srcp_t = const.tile([P, tok // 16], dt.int16)
        nc.sync.dma_start(srcp_t[:], srcp[:])
        dstrel_t = const.tile([P, chunks * blocks], dt.int16)
        nc.sync.dma_start(dstrel_t[:], dstrel[:])
        maskrd_t = const.tile([P, chunks * blocks * 4], dt.float16)
        nc.sync.dma_start(maskrd_t[:], maskrd[:])
        iota_t = const.tile([P, cg * P], dt.int16)
        nc.gpsimd.iota(iota_t[:], pattern=[[0, cg], [1, P]], base=0,
                       channel_multiplier=0)
        Wl0T_t = const.tile([feat, hid], dt.float32)
        nc.sync.dma_start(Wl0T_t[:], Wl0T[:])
        Wr0T_t = const.tile([feat, hid], dt.float32)
        nc.sync.dma_start(Wr0T_t[:], Wr0T[:])
        We0Th_t = const.tile([emb, hid], dt.float32)
        nc.sync.dma_start(We0Th_t[:], We0Th[:])
        Wr1T_t = const.tile([hid, dout], dt.float16)
        nc.sync.dma_start(Wr1T_t[:], Wr1T[:])
        We1T_t = const.tile([hid, dout], dt.float16)
        nc.sync.dma_start(We1T_t[:], We1T[:])
        Wl1T_t = const.tile([hid, dout], dt.float16)
        nc.sync.dma_start(Wl1T_t[:], Wl1T[:])
        b0_t = const.tile([hid, 1], dt.float32)
        nc.sync.dma_start(b0_t[:], b0[:])
        be0_t = const.tile([hid, 1], dt.float32)
        nc.sync.dma_start(be0_t[:], be0[:])
        b1_t = const.tile([dout, 1], dt.float32)
        nc.sync.dma_start(b1_t[:], b1[:])
        i642_t = const.tile([P, hid], dt.float16)
        nc.sync.dma_start(i642_t[:], i642[:])
        ident = const.tile([P, P], dt.float32)
        from concourse.masks import make_identity
        make_identity(nc, ident[:])

        # persistent activations (features on partitions)
        e0T_t = const.tile([emb, nodes_per], dt.float16)
        hT_t = const.tile([hid, nodes_per], dt.float16)

        # =========== Phase 1: e0T = (emb0+emb1) @ (0.5 We0.T) + be0 ===========
        G = 512
        for g0 in range(0, nodes_per, G) if not SKIP12 else []:
            g1 = min(g0 + G, nodes_per)
            w = g1 - g0
            xe0 = sb.tile([emb, G], dt.float32, tag="xe0")
            nc.sync.dma_start(xe0[:, :w], xembT[:emb, g0:g1])
            xe1 = sb.tile([emb, G], dt.float32, tag="xe1")
            nc.sync.dma_start(xe1[:, :w], xembT[emb:, g0:g1])
            esum = sb.tile([emb, G], dt.float32, tag="esum")
            nc.vector.tensor_add(esum[:, :w], xe0[:, :w], xe1[:, :w])
            pe0 = pp.tile([emb, G], dt.float32, tag="big")
            nc.tensor.matmul(pe0[:hid, :w], lhsT=We0Th_t[:], rhs=esum[:, :w],
                             start=True, stop=True)
            nc.scalar.activation(e0T_t[:, g0:g1], pe0[:hid, :w],
                                 mybir.ActivationFunctionType.Identity,
                                 bias=be0_t[:])

        # =========== Phase 2: y0 = x @ Wl0.T (own nodes, node-major fp16) ====
        y0_own = dram.tile([nodes_per, hid], dt.float16)
        for g0 in range(0, nodes_per, G) if (PHASES >= 2 and not SKIP12) else []:
            g1 = min(g0 + G, nodes_per)
            w = g1 - g0
            xg = sb.tile([feat, G], dt.float32, tag="xg")
            nc.sync.dma_start(xg[:, :w], xT[:, g0:g1])
            py0 = pp.tile([hid, G], dt.float32, tag="big")
            nc.tensor.matmul(py0[:, :w], lhsT=Wl0T_t[:], rhs=xg[:, :w],
                             start=True, stop=True)
            y0g = sb.tile([hid, G], dt.float32, tag="y0g")
            nc.vector.tensor_copy(y0g[:, :w], py0[:, :w])
            # transpose each 128-node block to node-major and store
            for b0_ in range(0, w, P):
                b1_ = min(b0_ + P, w)
                bw = b1_ - b0_
                ptr = pp1.tile([P, hid], dt.float32, tag="aux")
                nc.tensor.transpose(ptr[:bw, :], y0g[:, b0_:b1_], ident[:hid, :hid])
                str_ = sb.tile([P, hid], dt.float16, tag="str")
                nc.vector.tensor_copy(str_[:bw, :], ptr[:bw, :])
                nc.sync.dma_start(y0_own[g0 + b0_:g0 + b1_, :], str_[:bw, :])

        # =========== Phase 3: AllGather y0 -> table [n//4, 4*hid] ============
        y0_full = dram.tile([n // 4, 4 * hid], dt.float16, addr_space="Shared")
        if PHASES < 3:
            pass
        elif NO_COLLECTIVE:
            nc.gpsimd.dma_start(y0_full[:nodes_per // 4, :],
                                y0_own[:].rearrange("(a b) f -> a (b f)", b=4))
        else:
            nc.gpsimd.collective_compute(
                "AllGather", mybir.AluOpType.bypass,
                replica_groups=[list(range(nc_n))],
                ins=[y0_own[:]], outs=[y0_full[:]],
            )
        y0_tab = y0_full[:]

        # =========== Phase 4: L0 aggregation + h assembly ====================
        y1_own = dram.tile([nodes_per, dout], dt.float32)
        EL0 = 4 * hid                                     # 256 fp16 per token

        for grp in range(ngroups) if PHASES >= 4 else []:
            blo = grp * gb
            bhi = min(blo + gb, blocks)
            nb = bhi - blo
            t0 = blo * slots
            ntok = nb * slots
            g = sb3.tile([P, cg * EL0], dt.float16, tag="g0")
            for k0 in range(0, ntok, 1024):
                k1 = min(k0 + 1024, ntok)
                nc.gpsimd.dma_gather(
                    out_ap=g[:, k0 // P * EL0:k1 // P * EL0].rearrange(
                        "p (c e) -> p c e", e=EL0),
                    in_ap=y0_tab,
                    idxs_ap=srcp_t[:, (t0 + k0) // 16:(t0 + k1) // 16],
                    num_idxs=k1 - k0, num_idxs_reg=k1 - k0, elem_size=EL0,
                )
            # scale by mask*rdeg (zero 3 of 4 packed rows + 1/deg)
            if P4LEVEL < 2:
                continue
            g4 = g[:, :nb * chunks * EL0].rearrange("p (q f) -> p q f", f=hid)
            mr = maskrd_t[:, blo * chunks * 4:(blo * chunks + nb * chunks) * 4]
            nc.vector.tensor_tensor(
                out=g4, in0=g4,
                in1=mr.unsqueeze(2).broadcast_to([P, nb * chunks * 4, hid]),
                op=mybir.AluOpType.mult)
            # one-hot S for all chunks in group
            if P4LEVEL < 3:
                continue
            S = sb3.tile([P, cg * P], dt.float16, tag="S0")
            dr = dstrel_t[:, blo * chunks:blo * chunks + nb * chunks]
            nc.vector.tensor_tensor(
                out=S[:, :nb * chunks * P].rearrange("p (c q) -> p c q", q=P),
                in0=iota_t[:, :nb * chunks * P].rearrange("p (c q) -> p c q", q=P),
                in1=dr.unsqueeze(2).broadcast_to([P, nb * chunks, P]),
                op=mybir.AluOpType.is_equal)

            for lb in range(nb) if P4LEVEL >= 4 else []:
                b = blo + lb
                bw = tail if b == blocks - 1 else P
                # accumulate stacked pairs: psum[128, nodes]
                pag = pp.tile([P, P], dt.float32, tag="pag")
                for c in range(chunks):
                    cc = lb * chunks + c
                    for h2 in range(2):
                        nc.tensor.matmul(
                            pag[:, :bw],
                            lhsT=g[:, (cc * 4 + h2 * 2) * hid:(cc * 4 + h2 * 2 + 2) * hid],
                            rhs=S[:, cc * P:cc * P + bw],
                            start=(c == 0 and h2 == 0),
                            stop=(c == chunks - 1 and h2 == 1))
                if P4LEVEL < 5:
                    continue
                aggP = sb.tile([P, P], dt.float16, tag="aggP")
                nc.vector.tensor_copy(aggP[:, :bw], pag[:, :bw])
                xb = sb.tile([feat, P], dt.float32, tag="xb")
                nc.sync.dma_start(xb[:, :bw], xT[:, b * P:b * P + bw])
                ph = pp1.tile([hid, P], dt.float32, tag="aux")
                nc.tensor.matmul(ph[:, :bw], lhsT=Wr0T_t[:], rhs=xb[:, :bw],
                                 start=True, stop=False)
                nc.tensor.matmul(ph[:, :bw], lhsT=i642_t[:], rhs=aggP[:, :bw],
                                 start=False, stop=True)
                # h = relu(agg + xWr0 + e0 + bl0)
                hsum = sb.tile([hid, P], dt.float32, tag="hsum")
                nc.vector.tensor_add(hsum[:, :bw], ph[:, :bw],
                                     e0T_t[:, b * P:b * P + bw])
                nc.scalar.activation(hT_t[:, b * P:b * P + bw], hsum[:, :bw],
                                     mybir.ActivationFunctionType.Relu,
                                     bias=b0_t[:])
                if P4LEVEL < 6:
                    continue
                # y1 = h @ Wl1.T -> node-major fp32
                py1 = pp1.tile([dout, P], dt.float32, tag="aux")
                nc.tensor.matmul(py1[:, :bw], lhsT=Wl1T_t[:],
                                 rhs=hT_t[:, b * P:b * P + bw],
                                 start=True, stop=True)
                y1sb = sb.tile([dout, P], dt.float32, tag="y1sb")
                nc.vector.tensor_copy(y1sb[:, :bw], py1[:, :bw])
                ptr1 = pp1.tile([P, dout], dt.float32, tag="aux")
                nc.tensor.transpose(ptr1[:bw, :], y1sb[:, :bw], ident[:dout, :dout])
                y1tr = sb.tile([P, dout], dt.float32, tag="y1tr")
                nc.vector.tensor_copy(y1tr[:bw, :], ptr1[:bw, :])
                nc.sync.dma_start(y1_own[b * P:b * P + bw, :], y1tr[:bw, :])

        # =========== Phase 5: AllGather y1 ====================================
        y1_full = dram.tile([n // 4, 4 * dout], dt.float32, addr_space="Shared")
        if PHASES < 5:
            pass
        elif NO_COLLECTIVE:
            nc.gpsimd.dma_start(y1_full[:nodes_per // 4, :],
                                y1_own[:].rearrange("(a b) f -> a (b f)", b=4))
        else:
            nc.gpsimd.collective_compute(
                "AllGather", mybir.AluOpType.bypass,
                replica_groups=[list(range(nc_n))],
                ins=[y1_own[:]], outs=[y1_full[:]],
            )
        y1_tab = y1_full[:]

        # =========== Phase 6: L1 aggregation + output ========================
        EL1 = 4 * dout                                    # 64 fp32 per token
        if PHASES < 6:
            zz = const.tile([P, dout], dt.float32)
            nc.gpsimd.memset(zz[:], 0.0)
            for b in range(blocks):
                bw = tail if b == blocks - 1 else P
                nc.sync.dma_start(out_own[b * P:b * P + bw, :], zz[:bw, :])
        for grp in range(ngroups) if PHASES >= 6 else []:
            blo = grp * gb
            bhi = min(blo + gb, blocks)
            nb = bhi - blo
            t0 = blo * slots
            ntok = nb * slots
            g = sb3.tile([P, cg * EL1], dt.float32, tag="g1")
            for k0 in range(0, ntok, 1024):
                k1 = min(k0 + 1024, ntok)
                nc.gpsimd.dma_gather(
                    out_ap=g[:, k0 // P * EL1:k1 // P * EL1].rearrange(
                        "p (c e) -> p c e", e=EL1),
                    in_ap=y1_tab,
                    idxs_ap=srcp_t[:, (t0 + k0) // 16:(t0 + k1) // 16],
                    num_idxs=k1 - k0, num_idxs_reg=k1 - k0, elem_size=EL1,
                )
            g16 = sb3.tile([P, cg * EL1], dt.float16, tag="g16")
            mr = maskrd_t[:, blo * chunks * 4:(blo * chunks + nb * chunks) * 4]
            nc.vector.tensor_tensor(
                out=g16[:, :nb * chunks * EL1].rearrange("p (q f) -> p q f", f=dout),
                in0=g[:, :nb * chunks * EL1].rearrange("p (q f) -> p q f", f=dout),
                in1=mr.unsqueeze(2).broadcast_to([P, nb * chunks * 4, dout]),
                op=mybir.AluOpType.mult)
            # masked sum over the 4 packed rows selects the true message
            nct = nb * chunks
            gv = g16[:, :nct * EL1].rearrange("p (c q f) -> p c q f", q=4, f=dout)
            gs = sb3.tile([P, cg * dout], dt.float16, tag="gs")
            gsv = gs[:, :nct * dout].rearrange("p (c f) -> p c f", f=dout)
            nc.vector.tensor_add(gsv, gv[:, :, 0, :], gv[:, :, 1, :])
            gs2 = sb3.tile([P, cg * dout], dt.float16, tag="gs2")
            gs2v = gs2[:, :nct * dout].rearrange("p (c f) -> p c f", f=dout)
            nc.vector.tensor_add(gs2v, gv[:, :, 2, :], gv[:, :, 3, :])
            nc.vector.tensor_add(gsv, gsv, gs2v)
            S = sb3.tile([P, cg * P], dt.float16, tag="S0")
            dr = dstrel_t[:, blo * chunks:blo * chunks + nb * chunks]
            nc.vector.tensor_tensor(
                out=S[:, :nb * chunks * P].rearrange("p (c q) -> p c q", q=P),
                in0=iota_t[:, :nb * chunks * P].rearrange("p (c q) -> p c q", q=P),
                in1=dr.unsqueeze(2).broadcast_to([P, nb * chunks, P]),
                op=mybir.AluOpType.is_equal)

            for lb in range(nb):
                b = blo + lb
                bw = tail if b == blocks - 1 else P
                pag = pp.tile([dout, P], dt.float32, tag="pag")
                for c in range(chunks):
                    cc = lb * chunks + c
                    nc.tensor.matmul(
                        pag[:, :bw],
                        lhsT=gs[:, cc * dout:(cc + 1) * dout],
                        rhs=S[:, cc * P:cc * P + bw],
                        start=(c == 0), stop=False)
                # dense terms: h@Wr1.T + e0@We1.T into same psum
                nc.tensor.matmul(pag[:, :bw], lhsT=Wr1T_t[:],
                                 rhs=hT_t[:, b * P:b * P + bw],
                                 start=False, stop=False)
                nc.tensor.matmul(pag[:, :bw], lhsT=We1T_t[:],
                                 rhs=e0T_t[:, b * P:b * P + bw],
                                 start=False, stop=True)
                oT = sb.tile([dout, P], dt.float32, tag="oT")
                nc.scalar.activation(oT[:, :bw], pag[:, :bw],
                                     mybir.ActivationFunctionType.Identity,
                                     bias=b1_t[:])
                ptro = pp1.tile([P, dout], dt.float32, tag="aux")
                nc.tensor.transpose(ptro[:bw, :], oT[:, :bw], ident[:dout, :dout])
                osb = sb.tile([P, dout], dt.float32, tag="osb")
                nc.vector.tensor_copy(osb[:bw, :], ptro[:bw, :])
                nc.sync.dma_start(out_own[b * P:b * P + bw, :], osb[:bw, :])

    nc.compile()
    return nc


def prep_inputs(cfg, x_feat, x_emb, edge_index,
                Wl0, bl0, Wr0, We0, be0, Wl1, bl1, Wr1, We1, be1):
    """Shard + layout inputs for each core. Pure indexing/layout (plus weight
    transposes and the bias-constant folds); all FLOPs stay on device."""
    n, nodes_per = cfg["n"], cfg["nodes_per"]
    blocks, slots, chunks = cfg["blocks"], cfg["slots"], cfg["chunks"]
    nc_n = cfg["nc_n"]
    tok = slots * blocks

    src, dst = np.asarray(edge_index[0]), np.asarray(edge_index[1])
    in_maps = []
    w_common = dict(
        Wl0T=np.ascontiguousarray(Wl0.T, np.float32),
        Wr0T=np.ascontiguousarray(Wr0.T, np.float32),
        We0Th=np.ascontiguousarray(We0.T * 0.5, np.float32),
        Wr1T=np.ascontiguousarray(Wr1.T).astype(np.float16),
        We1T=np.ascontiguousarray(We1.T).astype(np.float16),
        Wl1T=np.ascontiguousarray(Wl1.T).astype(np.float16),
        b0=np.asarray(bl0, np.float32).reshape(-1, 1),
        be0=np.asarray(be0, np.float32).reshape(-1, 1),
        b1=(np.asarray(bl1, np.float32) + np.asarray(be1, np.float32)).reshape(-1, 1),
        i642=np.tile(np.eye(64, dtype=np.float16), (2, 1)),
    )
    for c in range(nc_n):
        lo = c * nodes_per
        m = (dst >= lo) & (dst < lo + nodes_per)
        s_c, d_c = src[m], dst[m] - lo
        deg = np.bincount(d_c, minlength=nodes_per)
        rdeg = 1.0 / np.maximum(deg, 1.0)
        blk = d_c >> 7
        order = np.argsort(blk, kind="stable")
        s_c, d_c, blk = s_c[order], d_c[order], blk[order]
        cnt = np.bincount(blk, minlength=blocks)
        if cnt.max() > slots:
            raise OverflowError(int(cnt.max()))
        # slot id within each block
        starts = np.zeros(blocks, np.int64)
        starts[1:] = np.cumsum(cnt)[:-1]
        slot_in_blk = np.arange(len(d_c)) - starts[blk]
        t = blk * slots + slot_in_blk                 # global token id

        srcp = np.zeros(tok, np.int16)
        srcp[t] = (s_c >> 2).astype(np.int16)
        dstrel = np.full((P, chunks * blocks), -1, np.int16)
        dstrel[t % P, t // P] = (d_c & 127).astype(np.int16)
        maskrd = np.zeros((P, chunks * blocks, 4), np.float16)
        maskrd[t % P, t // P, s_c & 3] = rdeg[d_c].astype(np.float16)

        in_maps.append(dict(
            xT=np.ascontiguousarray(np.asarray(x_feat[lo:lo + nodes_per]).T, np.float32),
            xembT=np.ascontiguousarray(
                np.asarray(x_emb[lo:lo + nodes_per]).transpose(1, 2, 0).reshape(2 * cfg["emb"], nodes_per),
                np.float32),
            srcp=np.ascontiguousarray(np.tile(srcp.reshape(-1, 16).T, (8, 1))),
            dstrel=np.ascontiguousarray(dstrel),
            maskrd=np.ascontiguousarray(maskrd.reshape(P, -1)),
            **w_common,
        ))
    return in_maps


def kernel(**inputs):
    src_dst = np.asarray(inputs["edge_index"])
    dst = src_dst[1]
    # runtime-chosen slot count: max edges in any 128-dst block, rounded to 128
    mx = 0
    for c in range(NC_N):
        lo = c * NODES_PER
        m = (dst >= lo) & (dst < lo + NODES_PER)
        d_c = dst[m] - lo
        cnt = np.bincount(d_c >> 7, minlength=(NODES_PER + P - 1) // P)
        mx = max(mx, int(cnt.max()))
    slots = max(((mx + P - 1) // P) * P, P)
    cfg = default_cfg(slots)
    nc = build_nc(cfg)
    in_maps = prep_inputs(cfg, **inputs)
    res = run_bass_kernel_spmd(nc, in_maps, list(range(NC_N)))
    kernel.last_res = res
    return np.concatenate([res.results[c]["out_own"] for c in range(NC_N)], axis=0)

